# revision 12
# baseline (speedup 1.0000x reference)
"""AdaOctConv kernel for 8 TRN2 NeuronCores (Bass/Tile).

Distribution: core i handles sample pair g = i//4 (samples 2g, 2g+1) and row
quarter q = i%4 of both the hf (256x256) and lf (128x128) branches.

Device algorithm (validated in numpy against the reference):
- All image-additive biases (ada bias, conv bias) and the input-inorm mean
  offset are spatially constant per channel (reflect padding preserves
  constants), so they cancel in the final instance norm and are dropped.
- The input inorm reduces to a per-channel scale s = rsqrt(var+eps), folded
  into the dynamic grouped spatial-conv weights.
- The grouped pointwise conv is absorbed into the shared dense conv:
  W' = CW @ PW (per-sample), so the pipeline is
    grouped3x3(s-folded dyn weights) -> dense3x3(W') -> inorm -> lrelu.
- Both 3x3 convs run as 9 shifted f32r matmuls accumulating in PSUM.
- Instance-norm statistics are exchanged across the 4 cores of a sample
  group with small AllReduces ([128, 8] and [128, 4]).
"""

import functools
import sys

sys.path.insert(0, "/opt/trn_rl_repo")

import numpy as np

N_CORES = 8
B, C, SC, H, EPS = 4, 128, 512, 256, 1e-5
GROUPS = [[0, 1, 2, 3], [4, 5, 6, 7]]

# branch geometry: (out rows per quarter, width, padded width, n blocks)
BR_H = dict(orows=64, w=256, wp=258, nblk=4)
BR_L = dict(orows=32, w=128, wp=130, nblk=2)

LAST_EXEC_NS = None
DEBUG = False


def _build():
    from concourse import bacc, mybir, tile

    DT = mybir.dt.float32
    F32R = mybir.dt.float32r
    BF16 = mybir.dt.bfloat16
    AF = mybir.ActivationFunctionType
    ALU = mybir.AluOpType
    AX = mybir.AxisListType

    nc = bacc.Bacc("TRN2", target_bir_lowering=False, debug=False,
                   num_devices=N_CORES)

    # ---- params ----
    xh = nc.declare_dram_parameter("x_hf", [2, C, 68, 258], DT, isOutput=False)
    xl = nc.declare_dram_parameter("x_lf", [2, C, 36, 130], DT, isOutput=False)
    sph = nc.declare_dram_parameter("s_hf", [SC, 9, 2, 9], DT, isOutput=False)
    spl = nc.declare_dram_parameter("s_lf", [SC, 9, 2, 9], DT, isOutput=False)
    spw = {k: nc.declare_dram_parameter(f"spw_{k}", [9, 2, 4, C, C], DT,
                                        isOutput=False) for k in "hl"}
    pww = {k: nc.declare_dram_parameter(f"pww_{k}", [2, 4, C, C], DT,
                                        isOutput=False) for k in "hl"}

    cwt = {k: nc.declare_dram_parameter(f"cwT_{k}", [C, 9, C], DT,
                                        isOutput=False) for k in "hl"}
    pswap_d = nc.declare_dram_parameter("pswap", [C, C], DT, isOutput=False)
    cvec_d = nc.declare_dram_parameter("cvec", [C, 16], DT, isOutput=False)
    oh_d = nc.declare_dram_parameter("out_hf", [2, C, 64, 256], DT,
                                     isOutput=True)
    ol_d = nc.declare_dram_parameter("out_lf", [2, C, 32, 128], DT,
                                     isOutput=True)
    dbg = {}
    if DEBUG:
        dbg["d_ops"] = nc.declare_dram_parameter("d_ops", [C, 64, 256], BF16,
                                                  isOutput=True)
        for nm, shp in [("d_wsp0h", [C, 2, 9]), ("d_wsp1h", [C, 2, 9]),
                        ("d_wpw0h", [C, 2]), ("d_wpw1h", [C, 2]),
                        ("d_gstats", [C, 8]), ("d_lg", [C, 9, C]),
                        ("d_ld", [C, 9, C]), ("d_yb", [C, 18, 258]),
                        ("d_osum", [C, 32]), ("d_osq", [C, 32]),
                        ("d_gout", [C, 4]), ("d_dsp", [C, 2, 9]),
                        ("d_osp", [C, 2, 9]),]:
            dbg[nm] = nc.declare_dram_parameter(nm, shp, DT, isOutput=True)
    xin = {"h": xh, "l": xl}
    spin = {"h": sph, "l": spl}
    out_d = {"h": oh_d, "l": ol_d}
    geo = {"h": BR_H, "l": BR_L}

    with tile.TileContext(nc) as tc:
        with (
            tc.tile_pool(name="sb", bufs=1) as sb,
            tc.tile_pool(name="ps", bufs=1, space="PSUM") as psp,
            tc.tile_pool(name="dr", bufs=1, space="DRAM") as dr,
        ):
            # ---- constants / persistent ----
            pswap = sb.tile([C, C], DT)
            nc.gpsimd.dma_start(out=pswap[:], in_=pswap_d[:])
            cvec = sb.tile([C, 16], DT)
            nc.gpsimd.dma_start(out=cvec[:], in_=cvec_d[:])
            epst = sb.tile([C, 1], DT)
            nc.gpsimd.memset(epst[:], EPS)
            alphat = sb.tile([C, 1], DT)
            nc.gpsimd.memset(alphat[:], 0.2)
            cwt_t = {}
            for k in "hl":
                cwt_t[k] = sb.tile([C, 9, C], F32R, name=f"cwt_{k}")
                nc.gpsimd.dma_start(out=cwt_t[k][:], in_=cwt[k][:])

            PAR = cvec[:, 0:1]
            NPAR = cvec[:, 1:2]

            # =========== predictor (both samples, both branches) ===========
            wsp = {}
            wpw = {}
            for k in "hl":
                spch = []
                for c in range(4):
                    t = sb.tile([C, 9, 2, 9], DT, name=f"spch_{k}{c}")
                    nc.gpsimd.dma_start(
                        out=t[:], in_=spin[k].ap()[128 * c:128 * (c + 1)])
                    spch.append(t)
                cb0 = 6 if k == "h" else 10
                spbt = cvec[:, cb0:cb0 + 2]
                pwbt = cvec[:, cb0 + 2:cb0 + 4]

                ps_wsp = [psp.tile([C, 18], DT, name=f"ps_wsp{j}_{k}", tag="ps",
                                   bufs=6) for j in range(2)]
                for t in range(9):
                    wt = sb.tile([C, 2, 4, C], DT, name=f"spwt_{k}{t}",
                                 tag="spwstream", bufs=3)
                    nc.gpsimd.dma_start(out=wt[:], in_=spw[k].ap()[t].transpose([2, 0, 1, 3]))
                    for j in range(2):
                        for c in range(4):
                            nc.tensor.matmul(
                                ps_wsp[j][:],
                                wt[:, j, c, :],
                                spch[c][:, t],
                                start=(t == 0 and c == 0),
                                stop=(t == 8 and c == 3),
                            )
                # sm (style sum; tap 4 of the im2col = the original 3x3)
                smt = sb.tile([C, 2, 4], DT, name=f"smt_{k}")
                for c in range(4):
                    nc.vector.tensor_reduce(
                        smt[:, :, c], spch[c][:, 4], axis=AX.X, op=ALU.add)
                ps_pw = [psp.tile([C, 2], DT, name=f"ps_pw{j}_{k}", tag="ps",
                                  bufs=6) for j in range(2)]
                pwt = sb.tile([C, 2, 4, C], DT, name=f"pwwt_{k}")
                nc.gpsimd.dma_start(out=pwt[:], in_=pww[k][:].transpose([2, 0, 1, 3]))
                for j in range(2):
                    for c in range(4):
                        nc.tensor.matmul(
                            ps_pw[j][:], pwt[:, j, c, :], smt[:, :, c],
                            start=(c == 0), stop=(c == 3))
                for j in range(2):
                    w1 = sb.tile([C, 2, 9], DT, name=f"wsp{j}_{k}")
                    nc.scalar.activation(w1[:], ps_wsp[j][:], AF.Identity,
                                         bias=spbt[:, j:j + 1])
                    wsp[(k, j)] = w1
                    w2 = sb.tile([C, 2], DT, name=f"wpw{j}_{k}")
                    nc.scalar.activation(w2[:], ps_pw[j][:], AF.Identity,
                                         bias=pwbt[:, j:j + 1], scale=1.0 / 9.0)
                    wpw[(k, j)] = w2
                    if DEBUG and k == "h":
                        nc.gpsimd.dma_start(out=dbg[f"d_wsp{j}h"][:], in_=w1[:])
                        nc.gpsimd.dma_start(out=dbg[f"d_wpw{j}h"][:], in_=w2[:])

            # parity mixes (both samples at once), per branch
            dmix = {}
            omix = {}
            for k in "hl":
                d_sp = sb.tile([C, 2, 9], DT, name=f"dsp_{k}")
                t1 = sb.tile([C, 2, 9], DT, name=f"wbt1_{k}", tag="wbt", bufs=4)
                nc.vector.tensor_scalar(t1[:], wsp[(k, 0)][:], NPAR, None,
                                        op0=ALU.mult)
                t2 = sb.tile([C, 2, 9], DT, name=f"wbt2_{k}", tag="wbt", bufs=4)
                nc.vector.tensor_scalar(t2[:], wsp[(k, 1)][:], PAR, None,
                                        op0=ALU.mult)
                nc.vector.tensor_tensor(d_sp[:], t1[:], t2[:], op=ALU.add)
                cr = sb.tile([C, 2, 9], DT, name=f"crsp_{k}", tag="wbt", bufs=4)
                t3 = sb.tile([C, 2, 9], DT, name=f"wbt3_{k}", tag="wbt", bufs=4)
                nc.vector.tensor_scalar(t3[:], wsp[(k, 0)][:], PAR, None,
                                        op0=ALU.mult)
                t4 = sb.tile([C, 2, 9], DT, name=f"wbt4_{k}", tag="wbt", bufs=4)
                nc.vector.tensor_scalar(t4[:], wsp[(k, 1)][:], NPAR, None,
                                        op0=ALU.mult)
                nc.vector.tensor_tensor(cr[:], t3[:], t4[:], op=ALU.add)
                ps_o = psp.tile([C, 18], DT, name=f"ps_osp_{k}", tag="ps",
                                bufs=6)
                nc.tensor.matmul(ps_o[:], pswap[:], cr[:], start=True,
                                 stop=True)
                o_sp = sb.tile([C, 2, 9], DT, name=f"osp_{k}")
                nc.scalar.activation(o_sp[:], ps_o[:], AF.Copy)
                dmix[(k, "sp")] = d_sp
                omix[(k, "sp")] = o_sp
                if DEBUG and k == "h":
                    nc.gpsimd.dma_start(out=dbg["d_dsp"][:], in_=d_sp[:])
                    nc.gpsimd.dma_start(out=dbg["d_osp"][:], in_=o_sp[:])

                d_pw = sb.tile([C, 2], DT, name=f"dpw_{k}")
                u1 = sb.tile([C, 2], DT, name=f"wbu1_{k}", tag="wbu", bufs=4)
                nc.vector.tensor_scalar(u1[:], wpw[(k, 0)][:], NPAR, None,
                                        op0=ALU.mult)
                u2 = sb.tile([C, 2], DT, name=f"wbu2_{k}", tag="wbu", bufs=4)
                nc.vector.tensor_scalar(u2[:], wpw[(k, 1)][:], PAR, None,
                                        op0=ALU.mult)
                nc.vector.tensor_tensor(d_pw[:], u1[:], u2[:], op=ALU.add)
                crp = sb.tile([C, 2], DT, name=f"crpw_{k}", tag="wbu", bufs=4)
                u3 = sb.tile([C, 2], DT, name=f"wbu3_{k}", tag="wbu", bufs=4)
                nc.vector.tensor_scalar(u3[:], wpw[(k, 0)][:], PAR, None,
                                        op0=ALU.mult)
                u4 = sb.tile([C, 2], DT, name=f"wbu4_{k}", tag="wbu", bufs=4)
                nc.vector.tensor_scalar(u4[:], wpw[(k, 1)][:], NPAR, None,
                                        op0=ALU.mult)
                nc.vector.tensor_tensor(crp[:], u3[:], u4[:], op=ALU.add)
                # PW[cm, ci]: diag = w_pw[cm, cm&1], off-diag (cm, cm^1) =
                # w_pw[cm, 1-(cm&1)] = crp  (no partition swap here)
                dmix[(k, "pw")] = d_pw
                omix[(k, "pw")] = crp

            # =========== input stats (both jobs) ===========
            sqsc = sb.tile([C, 512], DT, name="sqscratch")
            ar1_in = sb.tile([C, 8], DT, name="ar1_in")
            for jb in range(2):
                for ki, k in enumerate("hl"):
                    g = geo[k]
                    nrows = g["orows"]
                    ntile = nrows // 16
                    w, wp = g["w"], g["wp"]
                    sparts = sb.tile([C, ntile], DT, name=f"sp_in_{jb}{k}")
                    nch = (16 * w) // 512
                    rpc = 512 // w  # rows per 512-col chunk
                    qparts = sb.tile([C, ntile * nch], DT,
                                     name=f"sq_in_{jb}{k}")
                    for ti in range(ntile):
                        st = sb.tile([C, 16, wp], DT, name=f"stat_{jb}{k}{ti}",
                                     tag="xb", bufs=2)
                        nc.gpsimd.dma_start(
                            out=st[:],
                            in_=xin[k].ap()[jb, :, 2 + 16 * ti:2 + 16 * (ti + 1), :])
                        nc.vector.tensor_reduce(
                            sparts[:, ti:ti + 1], st[:, :, 1:1 + w], axis=AX.XY,
                            op=ALU.add)
                        # squares in 512-element (rpc-row) chunks
                        for ch in range(nch):
                            nc.scalar.activation(
                                sqsc[:].rearrange("p (r c) -> p r c", c=w),
                                st[:, rpc * ch:rpc * (ch + 1), 1:1 + w],
                                AF.Square,
                                accum_out=qparts[:, ti * nch + ch:ti * nch + ch + 1])
                    col = jb * 4 + ki * 2
                    nc.vector.tensor_reduce(
                        ar1_in[:, col:col + 1], sparts[:], axis=AX.X, op=ALU.add)
                    nc.vector.tensor_reduce(
                        ar1_in[:, col + 1:col + 2], qparts[:], axis=AX.X,
                        op=ALU.add)

            # AR1
            ar1_bi = dr.tile([C, 8], DT, name="ar1_bi")
            ar1_bo = dr.tile([C, 8], DT, name="ar1_bo")
            nc.gpsimd.dma_start(out=ar1_bi[:], in_=ar1_in[:])
            nc.gpsimd.collective_compute(
                "AllReduce", ALU.add, replica_groups=GROUPS,
                ins=[ar1_bi[:].opt()], outs=[ar1_bo[:].opt()])
            gstats = sb.tile([C, 8], DT, name="gstats")
            nc.gpsimd.dma_start(out=gstats[:], in_=ar1_bo[:])
            if DEBUG:
                nc.gpsimd.dma_start(out=dbg["d_gstats"][:], in_=gstats[:])

            # =========== per job: weights, conv, out-stats, epilogue ======
            op_scr = {}
            for jb in range(2):
                for k in "hl":
                    g = geo[k]
                    op_scr[(jb, k)] = dr.tile(
                        [C, g["orows"], g["w"]], BF16, name=f"opscr_{jb}{k}")

            def weight_build(jb, k):
                g = geo[k]
                n_img = (4 * g["orows"]) * g["w"]  # full-image pixel count
                col = jb * 4 + (0 if k == "h" else 2)
                m = sb.tile([C, 1], DT, name=f"m_{jb}{k}", tag="wb1", bufs=8)
                nc.scalar.activation(m[:], gstats[:, col:col + 1], AF.Copy,
                                     scale=1.0 / n_img)
                ex2 = sb.tile([C, 1], DT, name=f"ex2_{jb}{k}", tag="wb1",
                              bufs=8)
                nc.scalar.activation(ex2[:], gstats[:, col + 1:col + 2],
                                     AF.Copy, scale=1.0 / n_img)
                var = sb.tile([C, 1], DT, name=f"var_{jb}{k}", tag="wb1",
                              bufs=8)
                nc.vector.tensor_tensor(var[:], m[:], m[:], op=ALU.mult)
                nc.vector.tensor_tensor(var[:], ex2[:], var[:], op=ALU.subtract)
                sd = sb.tile([C, 1], DT, name=f"sd_{jb}{k}", tag="wb1", bufs=8)
                nc.scalar.activation(sd[:], var[:], AF.Sqrt, bias=epst[:])
                s_in = sb.tile([C, 1], DT, name=f"sin_{jb}{k}", tag="wb1",
                               bufs=8)
                nc.vector.reciprocal(s_in[:], sd[:])

                # grouped lhsT
                vd = sb.tile([C, 9], DT, name=f"vd_{jb}{k}", tag="wb9", bufs=4)
                nc.vector.tensor_scalar_mul(vd[:], dmix[(k, "sp")][:, jb, :], s_in[:])
                vo = sb.tile([C, 9], DT, name=f"vo_{jb}{k}", tag="wb9", bufs=4)
                nc.vector.tensor_scalar_mul(vo[:], omix[(k, "sp")][:, jb, :], s_in[:])
                d1 = sb.tile([C, 9, C], DT, name=f"d1_{jb}{k}", tag="dsel",
                             bufs=2)
                nc.gpsimd.affine_select(
                    d1[:], vd[:].unsqueeze(2).broadcast_to([C, 9, C]),
                    pattern=[[0, 9], [1, C]], compare_op=ALU.is_equal,
                    fill=0.0, base=0, channel_multiplier=-1)
                d2 = sb.tile([C, 9, C], DT, name=f"d2_{jb}{k}", tag="dsel",
                             bufs=2)
                nc.gpsimd.affine_select(
                    d2[:], vo[:].unsqueeze(2).broadcast_to([C, 9, C]),
                    pattern=[[0, 9], [1, C]], compare_op=ALU.is_equal,
                    fill=0.0, base=0, channel_multiplier=-1)
                lg = sb.tile([C, 9, 64, 2], F32R, name=f"lg_{jb}{k}",
                             tag=f"lhsg_{k}", bufs=2)
                nc.vector.tensor_tensor(
                    lg[:],
                    d1[:].rearrange("p t (a b) -> p t a b", b=2),
                    d2[:].rearrange("p t (a b) -> p t a b", b=2)[:, :, :, ::-1],
                    op=ALU.add)
                lg = lg[:].rearrange("p t a b -> p t (a b)")

                # dense lhsT = (CW . PW) pre-transposed
                pd = dmix[(k, "pw")][:, jb:jb + 1]
                po = omix[(k, "pw")][:, jb:jb + 1]
                p1 = sb.tile([C, C], DT, name=f"p1_{jb}{k}", tag="psel", bufs=2)
                nc.gpsimd.affine_select(
                    p1[:], pd.broadcast_to([C, C]), pattern=[[1, C]],
                    compare_op=ALU.is_equal, fill=0.0, base=0,
                    channel_multiplier=-1)
                p2 = sb.tile([C, C], DT, name=f"p2_{jb}{k}", tag="psel", bufs=2)
                nc.gpsimd.affine_select(
                    p2[:], po.broadcast_to([C, C]), pattern=[[1, C]],
                    compare_op=ALU.is_equal, fill=0.0, base=0,
                    channel_multiplier=-1)
                pw_t = sb.tile([C, 64, 2], F32R, name=f"pwm_{jb}{k}",
                               tag="pwm", bufs=2)
                nc.vector.tensor_tensor(
                    pw_t[:],
                    p1[:].rearrange("p (a b) -> p a b", b=2),
                    p2[:].rearrange("p (a b) -> p a b", b=2)[:, :, ::-1],
                    op=ALU.add)
                ld = sb.tile([C, 9, C], F32R, name=f"ld_{jb}{k}",
                             tag=f"lhsd_{k}", bufs=2)
                for t in range(9):
                    ps_w = psp.tile([C, C], DT, name=f"psw_{jb}{k}{t}",
                                    tag="ps", bufs=6)
                    nc.tensor.matmul(
                        ps_w[:], pw_t[:].rearrange("p a b -> p (a b)"),
                        cwt_t[k][:, t, :], start=True, stop=True)
                    nc.scalar.activation(ld[:, t, :], ps_w[:], AF.Copy)
                if DEBUG and jb == 0 and k == "h":
                    nc.gpsimd.dma_start(out=dbg["d_lg"][:], in_=lg)
                    nc.gpsimd.dma_start(out=dbg["d_ld"][:], in_=ld[:])
                return lg, ld

            def conv_branch(jb, k, lg, ld, sumparts, sqparts):
                g = geo[k]
                w, wp, nblk = g["w"], g["wp"], g["nblk"]
                stage = None
                for blk in range(nblk):
                    xb = sb.tile([C, 20, wp], F32R, name=f"xb_{jb}{k}{blk}",
                                 tag="xb", bufs=2)
                    nc.gpsimd.dma_start(
                        out=xb[:],
                        in_=xin[k].ap()[jb, :, 16 * blk:16 * blk + 20, :])
                    yb = sb.tile([C, 18, wp], F32R, name=f"yb_{jb}{k}{blk}",
                                 tag="yb", bufs=2)
                    # grouped conv: 9 strips of 2 y-rows
                    for s in range(9):
                        ps_g = psp.tile([C, 2, w], DT, name=f"psg_{jb}{k}{blk}{s}",
                                        tag="ps", bufs=6)
                        for t in range(9):
                            dy, dx = divmod(t, 3)
                            dy -= 1
                            nc.tensor.matmul(
                                ps_g[:], lg[:, t, :],
                                xb[:, 2 * s + 1 + dy:2 * s + 3 + dy, dx:dx + w],
                                start=(t == 0), stop=(t == 8))
                        nc.scalar.activation(yb[:, 2 * s:2 * s + 2, 1:1 + w],
                                             ps_g[:], AF.Copy)
                        # reflect pad cols from computed y
                        nc.vector.tensor_copy(
                            yb[:, 2 * s:2 * s + 2, 0:wp:wp - 1],
                            yb[:, 2 * s:2 * s + 2, 2:w:w - 3])
                    # edge fixups (no-ops on interior cores via masks)
                    if blk == 0:
                        e1 = sb.tile([C, 1, wp], DT, name=f"e1_{jb}{k}",
                                     tag="edge", bufs=4)
                        nc.vector.tensor_scalar(e1[:], yb[:, 0:1, :],
                                                cvec[:, 3:4], None, op0=ALU.mult)
                        e2 = sb.tile([C, 1, wp], DT, name=f"e2_{jb}{k}",
                                     tag="edge", bufs=4)
                        nc.vector.tensor_scalar(e2[:], yb[:, 2:3, :],
                                                cvec[:, 2:3], None, op0=ALU.mult)
                        nc.vector.tensor_tensor(yb[:, 0:1, :], e1[:], e2[:],
                                                op=ALU.add)
                    if blk == nblk - 1:
                        e3 = sb.tile([C, 1, wp], DT, name=f"e3_{jb}{k}",
                                     tag="edge", bufs=4)
                        nc.vector.tensor_scalar(e3[:], yb[:, 17:18, :],
                                                cvec[:, 5:6], None, op0=ALU.mult)
                        e4 = sb.tile([C, 1, wp], DT, name=f"e4_{jb}{k}",
                                     tag="edge", bufs=4)
                        nc.vector.tensor_scalar(e4[:], yb[:, 15:16, :],
                                                cvec[:, 4:5], None, op0=ALU.mult)
                        nc.vector.tensor_tensor(yb[:, 17:18, :], e3[:], e4[:],
                                                op=ALU.add)
                    if DEBUG and jb == 0 and k == "h" and blk == 0:
                        nc.gpsimd.dma_start(out=dbg["d_yb"][:], in_=yb[:])
                    # dense conv: 8 strips of 2 out-rows
                    for d in range(8):
                        sidx = 8 * blk + d
                        if d % 4 == 0:
                            stage = sb.tile([C, 8, w], BF16,
                                            name=f"stg_{jb}{k}{blk}{d}",
                                            tag="stage", bufs=3)
                        ps_d = psp.tile([C, 2, w], DT, name=f"psd_{jb}{k}{blk}{d}",
                                        tag="ps", bufs=6)
                        for t in range(9):
                            dy, dx = divmod(t, 3)
                            dy -= 1
                            nc.tensor.matmul(
                                ps_d[:], ld[:, t, :],
                                yb[:, 2 * d + 1 + dy:2 * d + 3 + dy, dx:dx + w],
                                start=(t == 0), stop=(t == 8))
                        nc.scalar.activation(
                            stage[:, 2 * (d % 4):2 * (d % 4) + 2, :], ps_d[:],
                            AF.Copy, accum_out=sumparts[:, sidx:sidx + 1])
                        nc.scalar.activation(
                            sqsc[:, 0:2 * w], ps_d[:], AF.Square,
                            accum_out=sqparts[:, sidx:sidx + 1])
                        if d % 4 == 3:
                            r0 = 16 * blk + 2 * (d - 3)
                            nc.gpsimd.dma_start(
                                out=op_scr[(jb, k)][:, r0:r0 + 8, :],
                                in_=stage[:])

            gout = {}
            for jb in range(2):
                lgs = {}
                for k in "hl":
                    lgs[k] = weight_build(jb, k)
                sumparts = {k: sb.tile([C, geo[k]["nblk"] * 8], DT,
                                       name=f"osum_{jb}{k}") for k in "hl"}
                sqparts = {k: sb.tile([C, geo[k]["nblk"] * 8], DT,
                                      name=f"osq_{jb}{k}") for k in "hl"}
                for k in "hl":
                    conv_branch(jb, k, *lgs[k], sumparts[k], sqparts[k])
                # out-stats AR
                ar2_in = sb.tile([C, 4], DT, name=f"ar2in_{jb}")
                for ki, k in enumerate("hl"):
                    nc.vector.tensor_reduce(ar2_in[:, 2 * ki:2 * ki + 1],
                                            sumparts[k][:], axis=AX.X,
                                            op=ALU.add)
                    nc.vector.tensor_reduce(ar2_in[:, 2 * ki + 1:2 * ki + 2],
                                            sqparts[k][:], axis=AX.X,
                                            op=ALU.add)
                ar2_bi = dr.tile([C, 4], DT, name=f"ar2bi_{jb}")
                ar2_bo = dr.tile([C, 4], DT, name=f"ar2bo_{jb}")
                nc.gpsimd.dma_start(out=ar2_bi[:], in_=ar2_in[:])
                nc.gpsimd.collective_compute(
                    "AllReduce", ALU.add, replica_groups=GROUPS,
                    ins=[ar2_bi[:].opt()], outs=[ar2_bo[:].opt()])
                go = sb.tile([C, 4], DT, name=f"gout_{jb}")
                nc.gpsimd.dma_start(out=go[:], in_=ar2_bo[:])
                gout[jb] = go
                if DEBUG and jb == 0:
                    nc.gpsimd.dma_start(out=dbg["d_ops"][:], in_=op_scr[(0, "h")][:])
                    nc.gpsimd.dma_start(out=dbg["d_osum"][:], in_=sumparts["h"][:])
                    nc.gpsimd.dma_start(out=dbg["d_osq"][:], in_=sqparts["h"][:])
                    nc.gpsimd.dma_start(out=dbg["d_gout"][:], in_=go[:])

            # epilogues
            for jb in range(2):
                for ki, k in enumerate("hl"):
                    g = geo[k]
                    w = g["w"]
                    n_img = (4 * g["orows"]) * w
                    col = 2 * ki
                    m = sb.tile([C, 1], DT, name=f"em_{jb}{k}", tag="wb1",
                                bufs=8)
                    nc.scalar.activation(m[:], gout[jb][:, col:col + 1],
                                         AF.Copy, scale=1.0 / n_img)
                    ex2 = sb.tile([C, 1], DT, name=f"eex2_{jb}{k}", tag="wb1",
                                  bufs=8)
                    nc.scalar.activation(ex2[:], gout[jb][:, col + 1:col + 2],
                                         AF.Copy, scale=1.0 / n_img)
                    var = sb.tile([C, 1], DT, name=f"evar_{jb}{k}", tag="wb1",
                                  bufs=8)
                    nc.vector.tensor_tensor(var[:], m[:], m[:], op=ALU.mult)
                    nc.vector.tensor_tensor(var[:], ex2[:], var[:],
                                            op=ALU.subtract)
                    sd = sb.tile([C, 1], DT, name=f"esd_{jb}{k}", tag="wb1",
                                 bufs=8)
                    nc.scalar.activation(sd[:], var[:], AF.Sqrt, bias=epst[:])
                    sc = sb.tile([C, 1], DT, name=f"esc_{jb}{k}", tag="wb1",
                                 bufs=8)
                    nc.vector.reciprocal(sc[:], sd[:])
                    bi = sb.tile([C, 1], DT, name=f"ebi_{jb}{k}", tag="wb1",
                                 bufs=8)
                    nc.vector.tensor_tensor(bi[:], m[:], sc[:], op=ALU.mult)
                    nc.vector.tensor_scalar(bi[:], bi[:], -1.0, None,
                                            op0=ALU.mult)
                    nrows = g["orows"]
                    for r0 in range(0, nrows, 8):
                        ei = sb.tile([C, 8, w], BF16, name=f"ei_{jb}{k}{r0}",
                                     tag="epin", bufs=2)
                        nc.gpsimd.dma_start(out=ei[:],
                                            in_=op_scr[(jb, k)][:, r0:r0 + 8, :])
                        eo = sb.tile([C, 8, w], DT, name=f"eo_{jb}{k}{r0}",
                                     tag="epout", bufs=2)
                        nc.scalar.activation(eo[:], ei[:], AF.Prelu,
                                             bias=bi[:], scale=sc[:],
                                             alpha=alphat[:])
                        nc.gpsimd.dma_start(
                            out=out_d[k].ap()[jb, :, r0:r0 + 8, :], in_=eo[:])

    nc.finalize()
    return nc


@functools.lru_cache(maxsize=1)
def _graph():
    return _build()


def _prep_host(inputs):
    c_hf = np.asarray(inputs["c_hf"], np.float32)
    c_lf = np.asarray(inputs["c_lf"], np.float32)
    s_hf = np.asarray(inputs["s_hf"], np.float32)
    s_lf = np.asarray(inputs["s_lf"], np.float32)

    def wprep(sp_w, pw_w, sp_b, pw_b, cw):
        w = sp_w.reshape(C, 2, SC, 9)
        spw_a = np.ascontiguousarray(
            np.transpose(w.reshape(C, 2, 4, C, 9), (4, 1, 2, 3, 0)))
        pw = pw_w.reshape(C, 2, SC)
        pww_a = np.ascontiguousarray(
            np.transpose(pw.reshape(C, 2, 4, C), (1, 2, 3, 0)))
        spb_a = np.ascontiguousarray(sp_b.reshape(C, 2))
        pwb_a = np.ascontiguousarray(pw_b.reshape(C, 2))
        cwt_a = np.ascontiguousarray(
            cw.reshape(C, C, 9).transpose(1, 2, 0))
        return spw_a, pww_a, spb_a, pwb_a, cwt_a

    wh = wprep(np.asarray(inputs["kp_h_sp_w"], np.float32),
               np.asarray(inputs["kp_h_pw_w"], np.float32),
               np.asarray(inputs["kp_h_sp_b"], np.float32),
               np.asarray(inputs["kp_h_pw_b"], np.float32),
               np.asarray(inputs["conv_h_w"], np.float32))
    wl = wprep(np.asarray(inputs["kp_l_sp_w"], np.float32),
               np.asarray(inputs["kp_l_pw_w"], np.float32),
               np.asarray(inputs["kp_l_sp_b"], np.float32),
               np.asarray(inputs["kp_l_pw_b"], np.float32),
               np.asarray(inputs["conv_l_w"], np.float32))

    pswap = np.zeros((C, C), np.float32)
    for c in range(C):
        pswap[c ^ 1, c] = 1.0

    xpad_h = [np.pad(c_hf[b], ((0, 0), (2, 2), (1, 1)), "reflect")
              for b in range(B)]
    xpad_l = [np.pad(c_lf[b], ((0, 0), (2, 2), (1, 1)), "reflect")
              for b in range(B)]
    def s_im2col(s01):
        # s01 [2, SC, 3, 3] -> [SC, 9, 2, 9]: [ci, tap, b, pix]
        sp = np.stack([np.pad(s, ((0, 0), (1, 1), (1, 1)), "reflect")
                       for s in s01])  # [2, SC, 5, 5]
        out = np.empty((SC, 9, 2, 9), np.float32)
        for t in range(9):
            ty, tx = divmod(t, 3)
            out[:, t] = np.transpose(
                sp[:, :, ty:ty + 3, tx:tx + 3].reshape(2, SC, 9), (1, 0, 2))
        return np.ascontiguousarray(out)

    in_maps = []
    for i in range(N_CORES):
        g, q = i // 4, i % 4
        bs = (2 * g, 2 * g + 1)
        cvec = np.zeros((C, 16), np.float32)
        cvec[1::2, 0] = 1.0          # parity
        cvec[:, 1] = 1.0 - cvec[:, 0]
        cvec[:, 2] = 1.0 if q == 0 else 0.0
        cvec[:, 3] = 1.0 - cvec[:, 2]
        cvec[:, 4] = 1.0 if q == 3 else 0.0
        cvec[:, 5] = 1.0 - cvec[:, 4]
        cvec[:, 6:8] = wh[2]
        cvec[:, 8:10] = wh[3]
        cvec[:, 10:12] = wl[2]
        cvec[:, 12:14] = wl[3]
        m = {
            "x_hf": np.ascontiguousarray(
                np.stack([xpad_h[b][:, 64 * q:64 * q + 68, :] for b in bs])),
            "x_lf": np.ascontiguousarray(
                np.stack([xpad_l[b][:, 32 * q:32 * q + 36, :] for b in bs])),
            "s_hf": s_im2col(s_hf[list(bs)]),
            "s_lf": s_im2col(s_lf[list(bs)]),
            "spw_h": wh[0], "pww_h": wh[1], "cwT_h": wh[4],
            "spw_l": wl[0], "pww_l": wl[1], "cwT_l": wl[4],
            "pswap": pswap, "cvec": cvec,
        }
        in_maps.append(m)
    return in_maps


def kernel(_trace=False, **inputs):
    global LAST_EXEC_NS
    from concourse.bass_utils import run_bass_kernel_spmd

    nc = _graph()
    in_maps = _prep_host(inputs)
    res = run_bass_kernel_spmd(nc, in_maps, core_ids=list(range(N_CORES)),
                               trace=_trace)
    LAST_EXEC_NS = res.exec_time_ns

    oh = np.zeros((B, C, H, H), np.float32)
    ol = np.zeros((B, C, H // 2, H // 2), np.float32)
    for i in range(N_CORES):
        g, q = i // 4, i % 4
        r = res.results[i]
        for jb, b in enumerate((2 * g, 2 * g + 1)):
            oh[b, :, 64 * q:64 * (q + 1), :] = r["out_hf"][jb]
            ol[b, :, 32 * q:32 * (q + 1), :] = r["out_lf"][jb]
    return oh, ol


# revision 17
# speedup vs baseline: 1.1371x; 1.1371x over previous
"""AdaOctConv kernel for 8 TRN2 NeuronCores (Bass/Tile).

Distribution: core i handles sample pair g = i//4 (samples 2g, 2g+1) and row
quarter q = i%4 of both the hf (256x256) and lf (128x128) branches.

Device algorithm (validated in numpy against the reference):
- All image-additive biases and the input-inorm mean offset cancel in the
  final instance norm (reflect padding preserves per-channel constants).
- The input inorm reduces to a per-channel scale s = rsqrt(var+eps), folded
  into the dynamic grouped spatial-conv weights.
- The grouped pointwise conv is absorbed into the shared dense conv:
  W' = CW @ PW per sample, so the pipeline is
    grouped3x3(s-folded dyn weights) -> dense3x3(W') -> inorm -> lrelu.
- Both convs run as 9 shifted f32r matmuls accumulating in PSUM.
- Stats cross the 4-core sample group via small AllReduces; convs are
  ordered A-hf, B-hf, A-lf, B-lf so epilogues overlap later convs.
"""

import functools
import sys

sys.path.insert(0, "/opt/trn_rl_repo")

import numpy as np

N_CORES = 8
B, C, SC, H, EPS = 4, 128, 512, 256, 1e-5
GROUPS = [[0, 1, 2, 3], [4, 5, 6, 7]]

BR_H = dict(orows=64, w=256, wp=258, nblk=4)
BR_L = dict(orows=32, w=128, wp=130, nblk=2)

LAST_EXEC_NS = None


def _build():
    from concourse import bacc, mybir, tile

    DT = mybir.dt.float32
    F32R = mybir.dt.float32r
    BF16 = mybir.dt.bfloat16
    AF = mybir.ActivationFunctionType
    ALU = mybir.AluOpType
    AX = mybir.AxisListType

    nc = bacc.Bacc("TRN2", target_bir_lowering=False, debug=False,
                   num_devices=N_CORES)

    xh = nc.declare_dram_parameter("x_hf", [2, C, 68, 258], F32R,
                                   isOutput=False)
    xl = nc.declare_dram_parameter("x_lf", [2, C, 36, 130], F32R,
                                   isOutput=False)
    sph = nc.declare_dram_parameter("s_hf", [SC, 9, 2, 9], DT, isOutput=False)
    spl = nc.declare_dram_parameter("s_lf", [SC, 9, 2, 9], DT, isOutput=False)
    spw = {k: nc.declare_dram_parameter(f"spw_{k}", [9, 2, 4, C, C], DT,
                                        isOutput=False) for k in "hl"}
    pww = {k: nc.declare_dram_parameter(f"pww_{k}", [2, 4, C, C], DT,
                                        isOutput=False) for k in "hl"}
    cwt = {k: nc.declare_dram_parameter(f"cwT_{k}", [C, 9, C], F32R,
                                        isOutput=False) for k in "hl"}
    pswap_d = nc.declare_dram_parameter("pswap", [C, C], DT, isOutput=False)
    cvec_d = nc.declare_dram_parameter("cvec", [C, 16], DT, isOutput=False)
    oh_d = nc.declare_dram_parameter("out_hf", [2, C, 64, 256], DT,
                                     isOutput=True)
    ol_d = nc.declare_dram_parameter("out_lf", [2, C, 32, 128], DT,
                                     isOutput=True)
    xin = {"h": xh, "l": xl}
    spin = {"h": sph, "l": spl}
    out_d = {"h": oh_d, "l": ol_d}
    geo = {"h": BR_H, "l": BR_L}

    with tile.TileContext(nc) as tc:
        with (
            tc.tile_pool(name="sb", bufs=1) as sb,
            tc.tile_pool(name="ps", bufs=1, space="PSUM") as psp,
            tc.tile_pool(name="dr", bufs=1, space="DRAM") as dr,
        ):
            # ---- constants ----
            cvec = sb.tile([C, 16], DT)
            nc.sync.dma_start(out=cvec[:], in_=cvec_d[:])
            epst = sb.tile([C, 1], DT)
            nc.gpsimd.memset(epst[:], EPS)
            alphat = sb.tile([C, 1], DT)
            nc.gpsimd.memset(alphat[:], 0.2)
            cwt_t = {}
            for k in "hl":
                cwt_t[k] = sb.tile([C, 9, C], F32R, name=f"cwt_{k}")
                nc.sync.dma_start(out=cwt_t[k][:], in_=cwt[k][:])
            PAR = cvec[:, 0:1]
            NPAR = cvec[:, 1:2]
            pswap = sb.tile([C, C], DT, name="pswap")
            nc.sync.dma_start(out=pswap[:], in_=pswap_d[:])

            sqsc = sb.tile([C, 512], DT, name="sqscratch")
            op_scr = {}
            for jb in range(2):
                for k in "hl":
                    g = geo[k]
                    op_scr[(jb, k)] = dr.tile(
                        [C, g["orows"], g["w"]], BF16, name=f"opscr_{jb}{k}")

            # ---------- input stats for one job ----------
            def in_stats(jb):
                ar_in = sb.tile([C, 4], DT, name=f"ar1in_{jb}")
                for ki, k in enumerate("hl"):
                    g = geo[k]
                    w, wp = g["w"], g["wp"]
                    nrows = g["orows"]
                    ntile = nrows // 8
                    rpc = 512 // w
                    nch = (8 * w) // 512
                    sparts = sb.tile([C, ntile], DT, name=f"sp_in_{jb}{k}")
                    qparts = sb.tile([C, ntile * nch], DT,
                                     name=f"sq_in_{jb}{k}")
                    for ti in range(ntile):
                        st = sb.tile([C, 8, wp], F32R,
                                     name=f"stat_{jb}{k}{ti}", tag="st",
                                     bufs=2)
                        nc.sync.dma_start(
                            out=st[:],
                            in_=xin[k].ap()[jb, :, 2 + 8 * ti:2 + 8 * (ti + 1), :])
                        nc.vector.tensor_reduce(
                            sparts[:, ti:ti + 1], st[:, :, 1:1 + w],
                            axis=AX.XY, op=ALU.add)
                        for ch in range(nch):
                            nc.scalar.activation(
                                sqsc[:].rearrange("p (r c) -> p r c", c=w),
                                st[:, rpc * ch:rpc * (ch + 1), 1:1 + w],
                                AF.Square,
                                accum_out=qparts[:, ti * nch + ch:ti * nch + ch + 1])
                    col = 2 * ki
                    nc.vector.tensor_reduce(
                        ar_in[:, col:col + 1], sparts[:], axis=AX.X,
                        op=ALU.add)
                    nc.vector.tensor_reduce(
                        ar_in[:, col + 1:col + 2], qparts[:], axis=AX.X,
                        op=ALU.add)
                bi = dr.tile([C, 4], DT, name=f"ar1bi_{jb}")
                bo = dr.tile([C, 4], DT, name=f"ar1bo_{jb}")
                nc.gpsimd.dma_start(out=bi[:], in_=ar_in[:])
                nc.gpsimd.collective_compute(
                    "AllReduce", ALU.add, replica_groups=GROUPS,
                    ins=[bi[:].opt()], outs=[bo[:].opt()])
                gs = sb.tile([C, 4], DT, name=f"gstats_{jb}")
                nc.gpsimd.dma_start(out=gs[:], in_=bo[:])
                return gs

            # ---------- predictor for one branch ----------
            wsp = {}
            wpw = {}

            def predictor(k):
                spch = []
                for c in range(4):
                    t = sb.tile([C, 9, 2, 9], DT, name=f"spch_{k}{c}")
                    nc.sync.dma_start(
                        out=t[:], in_=spin[k].ap()[128 * c:128 * (c + 1)])
                    spch.append(t)
                cb0 = 6 if k == "h" else 10
                spbt = cvec[:, cb0:cb0 + 2]
                pwbt = cvec[:, cb0 + 2:cb0 + 4]
                ps_wsp = [psp.tile([C, 18], DT, name=f"ps_wsp{j}_{k}",
                                   tag="ps", bufs=6) for j in range(2)]
                for t in range(9):
                    wt = sb.tile([C, 2, 4, C], DT, name=f"spwt_{k}{t}",
                                 tag="spwstream", bufs=2)
                    nc.sync.dma_start(
                        out=wt[:], in_=spw[k].ap()[t].transpose([2, 0, 1, 3]))
                    for j in range(2):
                        for c in range(4):
                            nc.tensor.matmul(
                                ps_wsp[j][:], wt[:, j, c, :], spch[c][:, t],
                                start=(t == 0 and c == 0),
                                stop=(t == 8 and c == 3))
                smt = sb.tile([C, 2, 4], DT, name=f"smt_{k}")
                for c in range(4):
                    nc.vector.tensor_reduce(
                        smt[:, :, c], spch[c][:, 4], axis=AX.X, op=ALU.add)
                ps_pw = [psp.tile([C, 2], DT, name=f"ps_pw{j}_{k}",
                                  tag="ps", bufs=6) for j in range(2)]
                pwt = sb.tile([C, 2, 4, C], DT, name=f"pwwt_{k}",
                              tag="spwstream", bufs=2)
                nc.sync.dma_start(out=pwt[:],
                                  in_=pww[k][:].transpose([2, 0, 1, 3]))
                for j in range(2):
                    for c in range(4):
                        nc.tensor.matmul(
                            ps_pw[j][:], pwt[:, j, c, :], smt[:, :, c],
                            start=(c == 0), stop=(c == 3))
                for j in range(2):
                    w1 = sb.tile([C, 2, 9], DT, name=f"wsp{j}_{k}")
                    nc.scalar.activation(w1[:], ps_wsp[j][:], AF.Identity,
                                         bias=spbt[:, j:j + 1])
                    wsp[(k, j)] = w1
                    w2 = sb.tile([C, 2], DT, name=f"wpw{j}_{k}")
                    nc.scalar.activation(w2[:], ps_pw[j][:], AF.Identity,
                                         bias=pwbt[:, j:j + 1], scale=1.0 / 9.0)
                    wpw[(k, j)] = w2

            # ---------- pre-AR weight prep (no stats dependency) ----------
            dmix = {}
            omix = {}

            def mixes(k):
                def par_mix(src0, src1, a0, a1, nm, shape, tag):
                    t1 = sb.tile(shape, DT, name=f"{nm}_t1", tag=tag, bufs=4)
                    nc.vector.tensor_scalar_mul(t1[:], src0, a0)
                    t2 = sb.tile(shape, DT, name=f"{nm}_t2", tag=tag, bufs=4)
                    nc.vector.tensor_scalar_mul(t2[:], src1, a1)
                    out = sb.tile(shape, DT, name=nm)
                    nc.vector.tensor_tensor(out[:], t1[:], t2[:], op=ALU.add)
                    return out

                dmix[(k, "sp")] = par_mix(wsp[(k, 0)][:], wsp[(k, 1)][:],
                                          NPAR, PAR, f"dsp_{k}", [C, 2, 9],
                                          "wbt")
                cr = par_mix(wsp[(k, 0)][:], wsp[(k, 1)][:], PAR, NPAR,
                             f"crsp_{k}", [C, 2, 9], "wbt")
                ps_o = psp.tile([C, 18], DT, name=f"ps_osp_{k}", tag="ps",
                                bufs=6)
                nc.tensor.matmul(ps_o[:], pswap[:], cr[:], start=True,
                                 stop=True)
                o_sp = sb.tile([C, 2, 9], DT, name=f"osp_{k}")
                nc.scalar.activation(o_sp[:], ps_o[:], AF.Copy)
                omix[(k, "sp")] = o_sp
                dmix[(k, "pw")] = par_mix(wpw[(k, 0)][:], wpw[(k, 1)][:],
                                          NPAR, PAR, f"dpw_{k}", [C, 2],
                                          "wbu")
                omix[(k, "pw")] = par_mix(wpw[(k, 0)][:], wpw[(k, 1)][:],
                                          PAR, NPAR, f"crpw_{k}", [C, 2],
                                          "wbu")

            lgt = {}
            ldt = {}

            def weight_prep(jb, k):
                # grouped lhsT, unscaled (scale by s_in post-AR, in place)
                d1 = sb.tile([C, 9, C], DT, name=f"d1_{jb}{k}", tag="dsel",
                             bufs=2)
                nc.gpsimd.affine_select(
                    d1[:], dmix[(k, "sp")][:, jb, :].unsqueeze(2)
                    .broadcast_to([C, 9, C]),
                    pattern=[[0, 9], [1, C]], compare_op=ALU.is_equal,
                    fill=0.0, base=0, channel_multiplier=-1)
                d2 = sb.tile([C, 9, C], DT, name=f"d2_{jb}{k}", tag="dsel",
                             bufs=2)
                nc.gpsimd.affine_select(
                    d2[:], omix[(k, "sp")][:, jb, :].unsqueeze(2)
                    .broadcast_to([C, 9, C]),
                    pattern=[[0, 9], [1, C]], compare_op=ALU.is_equal,
                    fill=0.0, base=0, channel_multiplier=-1)
                lg = sb.tile([C, 9, 64, 2], F32R, name=f"lg_{jb}{k}",
                             tag=f"lhsg_{k}", bufs=2)
                nc.vector.tensor_tensor(
                    lg[:],
                    d1[:].rearrange("p t (a b) -> p t a b", b=2),
                    d2[:].rearrange("p t (a b) -> p t a b", b=2)[:, :, :, ::-1],
                    op=ALU.add)
                lgt[(jb, k)] = lg

                # dense lhsT = (CW . PW) pre-transposed; PW off-diag is the
                # unswapped cross mix
                p1 = sb.tile([C, C], DT, name=f"p1_{jb}{k}", tag="psel",
                             bufs=2)
                nc.gpsimd.affine_select(
                    p1[:], dmix[(k, "pw")][:, jb:jb + 1].broadcast_to([C, C]),
                    pattern=[[1, C]], compare_op=ALU.is_equal, fill=0.0,
                    base=0, channel_multiplier=-1)
                p2 = sb.tile([C, C], DT, name=f"p2_{jb}{k}", tag="psel",
                             bufs=2)
                nc.gpsimd.affine_select(
                    p2[:], omix[(k, "pw")][:, jb:jb + 1].broadcast_to([C, C]),
                    pattern=[[1, C]], compare_op=ALU.is_equal, fill=0.0,
                    base=0, channel_multiplier=-1)
                pw_t = sb.tile([C, 64, 2], F32R, name=f"pwm_{jb}{k}",
                               tag="pwm", bufs=2)
                nc.vector.tensor_tensor(
                    pw_t[:],
                    p1[:].rearrange("p (a b) -> p a b", b=2),
                    p2[:].rearrange("p (a b) -> p a b", b=2)[:, :, ::-1],
                    op=ALU.add)
                ld = sb.tile([C, 9, C], F32R, name=f"ld_{jb}{k}",
                             tag=f"lhsd_{k}", bufs=2)
                for t in range(9):
                    ps_w = psp.tile([C, C], DT, name=f"psw_{jb}{k}{t}",
                                    tag="ps", bufs=6)
                    nc.tensor.matmul(
                        ps_w[:], pw_t[:].rearrange("p a b -> p (a b)"),
                        cwt_t[k][:, t, :], start=True, stop=True)
                    nc.scalar.activation(ld[:, t, :], ps_w[:], AF.Copy)
                ldt[(jb, k)] = ld

            # ---------- post-AR finalize: s_in, scale lg in place ----------
            s_in_t = {}

            def finalize(jb, gs):
                for ki, k in enumerate("hl"):
                    g = geo[k]
                    n_img = (4 * g["orows"]) * g["w"]
                    col = 2 * ki
                    m = sb.tile([C, 1], DT, name=f"m_{jb}{k}", tag="wb1",
                                bufs=8)
                    nc.scalar.activation(m[:], gs[:, col:col + 1], AF.Copy,
                                         scale=1.0 / n_img)
                    ex2 = sb.tile([C, 1], DT, name=f"ex2_{jb}{k}", tag="wb1",
                                  bufs=8)
                    nc.scalar.activation(ex2[:], gs[:, col + 1:col + 2],
                                         AF.Copy, scale=1.0 / n_img)
                    var = sb.tile([C, 1], DT, name=f"var_{jb}{k}", tag="wb1",
                                  bufs=8)
                    nc.vector.tensor_tensor(var[:], m[:], m[:], op=ALU.mult)
                    nc.vector.tensor_tensor(var[:], ex2[:], var[:],
                                            op=ALU.subtract)
                    sd = sb.tile([C, 1], DT, name=f"sd_{jb}{k}", tag="wb1",
                                 bufs=8)
                    nc.scalar.activation(sd[:], var[:], AF.Sqrt, bias=epst[:])
                    s_in = sb.tile([C, 1], DT, name=f"sin_{jb}{k}", tag="wb1",
                                   bufs=8)
                    nc.vector.reciprocal(s_in[:], sd[:])
                    s_in_t[(jb, k)] = s_in
                    lg = lgt[(jb, k)]
                    nc.vector.tensor_scalar_mul(lg[:], lg[:], s_in[:])

            # ---------- conv for one (job, branch) ----------
            def conv_branch(jb, k):
                g = geo[k]
                w, wp, nblk = g["w"], g["wp"], g["nblk"]
                lg, ld = lgt[(jb, k)], ldt[(jb, k)]
                nstrip = nblk * 8
                sumparts = sb.tile([C, nstrip], DT, name=f"osum_{jb}{k}")
                sqparts = sb.tile([C, nstrip], DT, name=f"osq_{jb}{k}")
                stage = None
                for blk in range(nblk):
                    xb = sb.tile([C, 20, wp], F32R, name=f"xb_{jb}{k}{blk}",
                                 tag="xb", bufs=2)
                    nc.sync.dma_start(
                        out=xb[:],
                        in_=xin[k].ap()[jb, :, 16 * blk:16 * blk + 20, :])
                    yb = sb.tile([C, 18, wp], F32R, name=f"yb_{jb}{k}{blk}",
                                 tag="yb", bufs=2)
                    for s in range(9):
                        ps_g = psp.tile([C, 2, w], DT,
                                        name=f"psg_{jb}{k}{blk}{s}",
                                        tag="ps", bufs=6)
                        for t in range(9):
                            dy, dx = divmod(t, 3)
                            dy -= 1
                            nc.tensor.matmul(
                                ps_g[:], lg[:].rearrange(
                                    "p t a b -> p t (a b)")[:, t, :],
                                xb[:, 2 * s + 1 + dy:2 * s + 3 + dy,
                                   dx:dx + w],
                                start=(t == 0), stop=(t == 8))
                        nc.scalar.activation(yb[:, 2 * s:2 * s + 2, 1:1 + w],
                                             ps_g[:], AF.Copy)
                        nc.vector.tensor_copy(
                            yb[:, 2 * s:2 * s + 2, 0:wp:wp - 1],
                            yb[:, 2 * s:2 * s + 2, 2:w:w - 3])
                    if blk == 0:
                        e1 = sb.tile([C, 1, wp], DT, name=f"e1_{jb}{k}",
                                     tag="edge", bufs=2)
                        nc.vector.tensor_scalar_mul(e1[:], yb[:, 0:1, :],
                                                    cvec[:, 3:4])
                        e2 = sb.tile([C, 1, wp], DT, name=f"e2_{jb}{k}",
                                     tag="edge", bufs=2)
                        nc.vector.tensor_scalar_mul(e2[:], yb[:, 2:3, :],
                                                    cvec[:, 2:3])
                        nc.vector.tensor_tensor(yb[:, 0:1, :], e1[:], e2[:],
                                                op=ALU.add)
                    if blk == nblk - 1:
                        e3 = sb.tile([C, 1, wp], DT, name=f"e3_{jb}{k}",
                                     tag="edge", bufs=2)
                        nc.vector.tensor_scalar_mul(e3[:], yb[:, 17:18, :],
                                                    cvec[:, 5:6])
                        e4 = sb.tile([C, 1, wp], DT, name=f"e4_{jb}{k}",
                                     tag="edge", bufs=2)
                        nc.vector.tensor_scalar_mul(e4[:], yb[:, 15:16, :],
                                                    cvec[:, 4:5])
                        nc.vector.tensor_tensor(yb[:, 17:18, :], e3[:], e4[:],
                                                op=ALU.add)
                    for d in range(8):
                        sidx = 8 * blk + d
                        if d % 4 == 0:
                            stage = sb.tile([C, 8, w], BF16,
                                            name=f"stg_{jb}{k}{blk}{d}",
                                            tag="stage", bufs=3)
                        ps_d = psp.tile([C, 2, w], DT,
                                        name=f"psd_{jb}{k}{blk}{d}",
                                        tag="ps", bufs=6)
                        for t in range(9):
                            dy, dx = divmod(t, 3)
                            dy -= 1
                            nc.tensor.matmul(
                                ps_d[:], ld[:, t, :],
                                yb[:, 2 * d + 1 + dy:2 * d + 3 + dy,
                                   dx:dx + w],
                                start=(t == 0), stop=(t == 8))
                        nc.scalar.activation(
                            stage[:, 2 * (d % 4):2 * (d % 4) + 2, :],
                            ps_d[:], AF.Copy,
                            accum_out=sumparts[:, sidx:sidx + 1])
                        nc.scalar.activation(
                            sqsc[:, 0:2 * w], ps_d[:], AF.Square,
                            accum_out=sqparts[:, sidx:sidx + 1])
                        if d % 4 == 3:
                            r0 = 16 * blk + 2 * (d - 3)
                            nc.sync.dma_start(
                                out=op_scr[(jb, k)][:, r0:r0 + 8, :],
                                in_=stage[:])
                # out-stats AR for this (job, branch)
                ar2 = sb.tile([C, 2], DT, name=f"ar2in_{jb}{k}")
                nc.vector.tensor_reduce(ar2[:, 0:1], sumparts[:], axis=AX.X,
                                        op=ALU.add)
                nc.vector.tensor_reduce(ar2[:, 1:2], sqparts[:], axis=AX.X,
                                        op=ALU.add)
                bi = dr.tile([C, 2], DT, name=f"ar2bi_{jb}{k}")
                bo = dr.tile([C, 2], DT, name=f"ar2bo_{jb}{k}")
                nc.gpsimd.dma_start(out=bi[:], in_=ar2[:])
                nc.gpsimd.collective_compute(
                    "AllReduce", ALU.add, replica_groups=GROUPS,
                    ins=[bi[:].opt()], outs=[bo[:].opt()])
                go = sb.tile([C, 2], DT, name=f"gout_{jb}{k}")
                nc.gpsimd.dma_start(out=go[:], in_=bo[:])
                return go

            # ---------- epilogue ----------
            def epilogue(jb, k, go):
                g = geo[k]
                w = g["w"]
                n_img = (4 * g["orows"]) * w
                m = sb.tile([C, 1], DT, name=f"em_{jb}{k}", tag="wb1", bufs=8)
                nc.scalar.activation(m[:], go[:, 0:1], AF.Copy,
                                     scale=1.0 / n_img)
                ex2 = sb.tile([C, 1], DT, name=f"eex2_{jb}{k}", tag="wb1",
                              bufs=8)
                nc.scalar.activation(ex2[:], go[:, 1:2], AF.Copy,
                                     scale=1.0 / n_img)
                var = sb.tile([C, 1], DT, name=f"evar_{jb}{k}", tag="wb1",
                              bufs=8)
                nc.vector.tensor_tensor(var[:], m[:], m[:], op=ALU.mult)
                nc.vector.tensor_tensor(var[:], ex2[:], var[:],
                                        op=ALU.subtract)
                sd = sb.tile([C, 1], DT, name=f"esd_{jb}{k}", tag="wb1",
                             bufs=8)
                nc.scalar.activation(sd[:], var[:], AF.Sqrt, bias=epst[:])
                sc = sb.tile([C, 1], DT, name=f"esc_{jb}{k}", tag="wb1",
                             bufs=8)
                nc.vector.reciprocal(sc[:], sd[:])
                bi = sb.tile([C, 1], DT, name=f"ebi_{jb}{k}", tag="wb1",
                             bufs=8)
                nc.vector.tensor_tensor(bi[:], m[:], sc[:], op=ALU.mult)
                nc.vector.tensor_scalar_mul(bi[:], bi[:], -1.0)
                for r0 in range(0, g["orows"], 8):
                    ei = sb.tile([C, 8, w], BF16, name=f"ei_{jb}{k}{r0}",
                                 tag="epin", bufs=2)
                    nc.sync.dma_start(out=ei[:],
                                      in_=op_scr[(jb, k)][:, r0:r0 + 8, :])
                    eo = sb.tile([C, 8, w], DT, name=f"eo_{jb}{k}{r0}",
                                 tag="epout", bufs=2)
                    nc.scalar.activation(eo[:], ei[:], AF.Prelu, bias=bi[:],
                                         scale=sc[:], alpha=alphat[:])
                    nc.sync.dma_start(out=out_d[k].ap()[jb, :, r0:r0 + 8, :],
                                      in_=eo[:])

            # ================= schedule =================
            gsA = in_stats(0)
            predictor("h")
            predictor("l")
            for k in "hl":
                mixes(k)
            for jb in range(2):
                for k in "hl":
                    weight_prep(jb, k)
            finalize(0, gsA)
            gsB = in_stats(1)
            finalize(1, gsB)
            gout = {}
            gout[(0, "h")] = conv_branch(0, "h")
            gout[(1, "h")] = conv_branch(1, "h")
            gout[(0, "l")] = conv_branch(0, "l")
            gout[(1, "l")] = conv_branch(1, "l")
            for jb, k in [(0, "h"), (1, "h"), (0, "l"), (1, "l")]:
                epilogue(jb, k, gout[(jb, k)])

    nc.finalize()
    return nc


@functools.lru_cache(maxsize=1)
def _graph():
    return _build()


def _prep_host(inputs):
    c_hf = np.asarray(inputs["c_hf"], np.float32)
    c_lf = np.asarray(inputs["c_lf"], np.float32)
    s_hf = np.asarray(inputs["s_hf"], np.float32)
    s_lf = np.asarray(inputs["s_lf"], np.float32)

    def wprep(sp_w, pw_w, sp_b, pw_b, cw):
        w = sp_w.reshape(C, 2, SC, 9)
        spw_a = np.ascontiguousarray(
            np.transpose(w.reshape(C, 2, 4, C, 9), (4, 1, 2, 3, 0)))
        pw = pw_w.reshape(C, 2, SC)
        pww_a = np.ascontiguousarray(
            np.transpose(pw.reshape(C, 2, 4, C), (1, 2, 3, 0)))
        spb_a = np.ascontiguousarray(sp_b.reshape(C, 2))
        pwb_a = np.ascontiguousarray(pw_b.reshape(C, 2))
        cwt_a = np.ascontiguousarray(
            cw.reshape(C, C, 9).transpose(1, 2, 0))
        return spw_a, pww_a, spb_a, pwb_a, cwt_a

    wh = wprep(np.asarray(inputs["kp_h_sp_w"], np.float32),
               np.asarray(inputs["kp_h_pw_w"], np.float32),
               np.asarray(inputs["kp_h_sp_b"], np.float32),
               np.asarray(inputs["kp_h_pw_b"], np.float32),
               np.asarray(inputs["conv_h_w"], np.float32))
    wl = wprep(np.asarray(inputs["kp_l_sp_w"], np.float32),
               np.asarray(inputs["kp_l_pw_w"], np.float32),
               np.asarray(inputs["kp_l_sp_b"], np.float32),
               np.asarray(inputs["kp_l_pw_b"], np.float32),
               np.asarray(inputs["conv_l_w"], np.float32))

    pswap = np.zeros((C, C), np.float32)
    for c in range(C):
        pswap[c ^ 1, c] = 1.0

    xpad_h = [np.pad(c_hf[b], ((0, 0), (2, 2), (1, 1)), "reflect")
              for b in range(B)]
    xpad_l = [np.pad(c_lf[b], ((0, 0), (2, 2), (1, 1)), "reflect")
              for b in range(B)]
    def s_im2col(s01):
        # s01 [2, SC, 3, 3] -> [SC, 9, 2, 9]: [ci, tap, b, pix]
        sp = np.stack([np.pad(s, ((0, 0), (1, 1), (1, 1)), "reflect")
                       for s in s01])  # [2, SC, 5, 5]
        out = np.empty((SC, 9, 2, 9), np.float32)
        for t in range(9):
            ty, tx = divmod(t, 3)
            out[:, t] = np.transpose(
                sp[:, :, ty:ty + 3, tx:tx + 3].reshape(2, SC, 9), (1, 0, 2))
        return np.ascontiguousarray(out)

    in_maps = []
    for i in range(N_CORES):
        g, q = i // 4, i % 4
        bs = (2 * g, 2 * g + 1)
        cvec = np.zeros((C, 16), np.float32)
        cvec[1::2, 0] = 1.0          # parity
        cvec[:, 1] = 1.0 - cvec[:, 0]
        cvec[:, 2] = 1.0 if q == 0 else 0.0
        cvec[:, 3] = 1.0 - cvec[:, 2]
        cvec[:, 4] = 1.0 if q == 3 else 0.0
        cvec[:, 5] = 1.0 - cvec[:, 4]
        cvec[:, 6:8] = wh[2]
        cvec[:, 8:10] = wh[3]
        cvec[:, 10:12] = wl[2]
        cvec[:, 12:14] = wl[3]
        m = {
            "x_hf": np.ascontiguousarray(
                np.stack([xpad_h[b][:, 64 * q:64 * q + 68, :] for b in bs])),
            "x_lf": np.ascontiguousarray(
                np.stack([xpad_l[b][:, 32 * q:32 * q + 36, :] for b in bs])),
            "s_hf": s_im2col(s_hf[list(bs)]),
            "s_lf": s_im2col(s_lf[list(bs)]),
            "spw_h": wh[0], "pww_h": wh[1], "cwT_h": wh[4],
            "spw_l": wl[0], "pww_l": wl[1], "cwT_l": wl[4],
            "pswap": pswap, "cvec": cvec,
        }
        in_maps.append(m)
    return in_maps


def kernel(_trace=False, **inputs):
    global LAST_EXEC_NS
    from concourse.bass_utils import run_bass_kernel_spmd

    nc = _graph()
    in_maps = _prep_host(inputs)
    res = run_bass_kernel_spmd(nc, in_maps, core_ids=list(range(N_CORES)),
                               trace=_trace)
    LAST_EXEC_NS = res.exec_time_ns

    oh = np.zeros((B, C, H, H), np.float32)
    ol = np.zeros((B, C, H // 2, H // 2), np.float32)
    for i in range(N_CORES):
        g, q = i // 4, i % 4
        r = res.results[i]
        for jb, b in enumerate((2 * g, 2 * g + 1)):
            oh[b, :, 64 * q:64 * (q + 1), :] = r["out_hf"][jb]
            ol[b, :, 32 * q:32 * (q + 1), :] = r["out_lf"][jb]
    return oh, ol


# revision 19
# speedup vs baseline: 1.1404x; 1.0028x over previous
"""AdaOctConv kernel for 8 TRN2 NeuronCores (Bass/Tile).

Distribution: core i handles sample pair g = i//4 (samples 2g, 2g+1) and row
quarter q = i%4 of both the hf (256x256) and lf (128x128) branches.

Device algorithm (validated in numpy against the reference):
- All image-additive biases and the input-inorm mean offset cancel in the
  final instance norm (reflect padding preserves per-channel constants).
- The input inorm reduces to a per-channel scale s = rsqrt(var+eps), folded
  into the dynamic grouped spatial-conv weights.
- The grouped pointwise conv is absorbed into the shared dense conv:
  W' = CW @ PW per sample, so the pipeline is
    grouped3x3(s-folded dyn weights) -> dense3x3(W') -> inorm -> lrelu.
- Both convs run as 9 shifted f32r matmuls accumulating in PSUM.
- Stats cross the 4-core sample group via small AllReduces; convs are
  ordered A-hf, B-hf, A-lf, B-lf so epilogues overlap later convs.
"""

import functools
import sys

sys.path.insert(0, "/opt/trn_rl_repo")

import numpy as np

N_CORES = 8
B, C, SC, H, EPS = 4, 128, 512, 256, 1e-5
GROUPS = [[0, 1, 2, 3], [4, 5, 6, 7]]

BR_H = dict(orows=64, w=256, wp=258, nblk=4)
BR_L = dict(orows=32, w=128, wp=130, nblk=2)

LAST_EXEC_NS = None


def _build():
    from concourse import bacc, mybir, tile

    DT = mybir.dt.float32
    F32R = mybir.dt.float32r
    BF16 = mybir.dt.bfloat16
    AF = mybir.ActivationFunctionType
    ALU = mybir.AluOpType
    AX = mybir.AxisListType

    nc = bacc.Bacc("TRN2", target_bir_lowering=False, debug=False,
                   num_devices=N_CORES)

    xh = nc.declare_dram_parameter("x_hf", [2, C, 68, 258], F32R,
                                   isOutput=False)
    xl = nc.declare_dram_parameter("x_lf", [2, C, 36, 130], F32R,
                                   isOutput=False)
    sph = nc.declare_dram_parameter("s_hf", [SC, 9, 2, 9], DT, isOutput=False)
    spl = nc.declare_dram_parameter("s_lf", [SC, 9, 2, 9], DT, isOutput=False)
    spw = {k: nc.declare_dram_parameter(f"spw_{k}", [9, 2, 4, C, C], DT,
                                        isOutput=False) for k in "hl"}
    pww = {k: nc.declare_dram_parameter(f"pww_{k}", [2, 4, C, C], DT,
                                        isOutput=False) for k in "hl"}
    cwt = {k: nc.declare_dram_parameter(f"cwT_{k}", [C, 9, C], F32R,
                                        isOutput=False) for k in "hl"}
    pswap_d = nc.declare_dram_parameter("pswap", [C, C], DT, isOutput=False)
    cvec_d = nc.declare_dram_parameter("cvec", [C, 16], DT, isOutput=False)
    oh_d = nc.declare_dram_parameter("out_hf", [2, C, 64, 256], DT,
                                     isOutput=True)
    ol_d = nc.declare_dram_parameter("out_lf", [2, C, 32, 128], DT,
                                     isOutput=True)
    xin = {"h": xh, "l": xl}
    spin = {"h": sph, "l": spl}
    out_d = {"h": oh_d, "l": ol_d}
    geo = {"h": BR_H, "l": BR_L}

    with tile.TileContext(nc) as tc:
        with (
            tc.tile_pool(name="sb", bufs=1) as sb,
            tc.tile_pool(name="ps", bufs=1, space="PSUM") as psp,
            tc.tile_pool(name="dr", bufs=1, space="DRAM") as dr,
        ):
            # ---- constants ----
            cvec = sb.tile([C, 16], DT)
            nc.sync.dma_start(out=cvec[:], in_=cvec_d[:])
            epst = sb.tile([C, 1], DT)
            nc.gpsimd.memset(epst[:], EPS)
            alphat = sb.tile([C, 1], DT)
            nc.gpsimd.memset(alphat[:], 0.2)
            cwt_t = {}
            for k in "hl":
                cwt_t[k] = sb.tile([C, 9, C], F32R, name=f"cwt_{k}")
                nc.sync.dma_start(out=cwt_t[k][:], in_=cwt[k][:])
            PAR = cvec[:, 0:1]
            NPAR = cvec[:, 1:2]
            pswap = sb.tile([C, C], DT, name="pswap")
            nc.sync.dma_start(out=pswap[:], in_=pswap_d[:])

            sqsc = sb.tile([C, 512], DT, name="sqscratch")
            op_scr = {}
            for jb in range(2):
                for k in "hl":
                    g = geo[k]
                    op_scr[(jb, k)] = dr.tile(
                        [C, g["orows"], g["w"]], BF16, name=f"opscr_{jb}{k}")

            # ---------- input stats for one job ----------
            def in_stats(jb):
                ar_in = sb.tile([C, 4], DT, name=f"ar1in_{jb}")
                for ki, k in enumerate("hl"):
                    g = geo[k]
                    w, wp = g["w"], g["wp"]
                    nrows = g["orows"]
                    ntile = nrows // 8
                    rpc = 512 // w
                    nch = (8 * w) // 512
                    sparts = sb.tile([C, ntile], DT, name=f"sp_in_{jb}{k}")
                    qparts = sb.tile([C, ntile * nch], DT,
                                     name=f"sq_in_{jb}{k}")
                    for ti in range(ntile):
                        st = sb.tile([C, 8, wp], F32R,
                                     name=f"stat_{jb}{k}{ti}", tag="st",
                                     bufs=2)
                        nc.sync.dma_start(
                            out=st[:],
                            in_=xin[k].ap()[jb, :, 2 + 8 * ti:2 + 8 * (ti + 1), :])
                        nc.vector.tensor_reduce(
                            sparts[:, ti:ti + 1], st[:, :, 1:1 + w],
                            axis=AX.XY, op=ALU.add)
                        for ch in range(nch):
                            nc.scalar.activation(
                                sqsc[:].rearrange("p (r c) -> p r c", c=w),
                                st[:, rpc * ch:rpc * (ch + 1), 1:1 + w],
                                AF.Square,
                                accum_out=qparts[:, ti * nch + ch:ti * nch + ch + 1])
                    col = 2 * ki
                    nc.vector.tensor_reduce(
                        ar_in[:, col:col + 1], sparts[:], axis=AX.X,
                        op=ALU.add)
                    nc.vector.tensor_reduce(
                        ar_in[:, col + 1:col + 2], qparts[:], axis=AX.X,
                        op=ALU.add)
                bi = dr.tile([C, 4], DT, name=f"ar1bi_{jb}")
                bo = dr.tile([C, 4], DT, name=f"ar1bo_{jb}")
                nc.gpsimd.dma_start(out=bi[:], in_=ar_in[:])
                nc.gpsimd.collective_compute(
                    "AllReduce", ALU.add, replica_groups=GROUPS,
                    ins=[bi[:].opt()], outs=[bo[:].opt()])
                gs = sb.tile([C, 4], DT, name=f"gstats_{jb}")
                nc.gpsimd.dma_start(out=gs[:], in_=bo[:])
                return gs

            # ---------- predictor for one branch ----------
            wsp = {}
            wpw = {}

            def predictor(k):
                spch = []
                for c in range(4):
                    t = sb.tile([C, 9, 2, 9], DT, name=f"spch_{k}{c}")
                    nc.sync.dma_start(
                        out=t[:], in_=spin[k].ap()[128 * c:128 * (c + 1)])
                    spch.append(t)
                cb0 = 6 if k == "h" else 10
                spbt = cvec[:, cb0:cb0 + 2]
                pwbt = cvec[:, cb0 + 2:cb0 + 4]
                ps_wsp = [psp.tile([C, 18], DT, name=f"ps_wsp{j}_{k}",
                                   tag="ps", bufs=7) for j in range(2)]
                for t in range(9):
                    wt = sb.tile([C, 2, 4, C], DT, name=f"spwt_{k}{t}",
                                 tag="spwstream", bufs=2)
                    nc.sync.dma_start(
                        out=wt[:], in_=spw[k].ap()[t].transpose([2, 0, 1, 3]))
                    for j in range(2):
                        for c in range(4):
                            nc.tensor.matmul(
                                ps_wsp[j][:], wt[:, j, c, :], spch[c][:, t],
                                start=(t == 0 and c == 0),
                                stop=(t == 8 and c == 3))
                smt = sb.tile([C, 2, 4], DT, name=f"smt_{k}")
                for c in range(4):
                    nc.vector.tensor_reduce(
                        smt[:, :, c], spch[c][:, 4], axis=AX.X, op=ALU.add)
                ps_pw = [psp.tile([C, 2], DT, name=f"ps_pw{j}_{k}",
                                  tag="ps", bufs=7) for j in range(2)]
                pwt = sb.tile([C, 2, 4, C], DT, name=f"pwwt_{k}",
                              tag="spwstream", bufs=2)
                nc.sync.dma_start(out=pwt[:],
                                  in_=pww[k][:].transpose([2, 0, 1, 3]))
                for j in range(2):
                    for c in range(4):
                        nc.tensor.matmul(
                            ps_pw[j][:], pwt[:, j, c, :], smt[:, :, c],
                            start=(c == 0), stop=(c == 3))
                for j in range(2):
                    w1 = sb.tile([C, 2, 9], DT, name=f"wsp{j}_{k}")
                    nc.scalar.activation(w1[:], ps_wsp[j][:], AF.Identity,
                                         bias=spbt[:, j:j + 1])
                    wsp[(k, j)] = w1
                    w2 = sb.tile([C, 2], DT, name=f"wpw{j}_{k}")
                    nc.scalar.activation(w2[:], ps_pw[j][:], AF.Identity,
                                         bias=pwbt[:, j:j + 1], scale=1.0 / 9.0)
                    wpw[(k, j)] = w2

            # ---------- pre-AR weight prep (no stats dependency) ----------
            dmix = {}
            omix = {}

            def mixes(k):
                def par_mix(src0, src1, a0, a1, nm, shape, tag):
                    t1 = sb.tile(shape, DT, name=f"{nm}_t1", tag=tag, bufs=4)
                    nc.vector.tensor_scalar_mul(t1[:], src0, a0)
                    t2 = sb.tile(shape, DT, name=f"{nm}_t2", tag=tag, bufs=4)
                    nc.vector.tensor_scalar_mul(t2[:], src1, a1)
                    out = sb.tile(shape, DT, name=nm)
                    nc.vector.tensor_tensor(out[:], t1[:], t2[:], op=ALU.add)
                    return out

                dmix[(k, "sp")] = par_mix(wsp[(k, 0)][:], wsp[(k, 1)][:],
                                          NPAR, PAR, f"dsp_{k}", [C, 2, 9],
                                          "wbt")
                cr = par_mix(wsp[(k, 0)][:], wsp[(k, 1)][:], PAR, NPAR,
                             f"crsp_{k}", [C, 2, 9], "wbt")
                ps_o = psp.tile([C, 18], DT, name=f"ps_osp_{k}", tag="ps",
                                bufs=7)
                nc.tensor.matmul(ps_o[:], pswap[:], cr[:], start=True,
                                 stop=True)
                o_sp = sb.tile([C, 2, 9], DT, name=f"osp_{k}")
                nc.scalar.activation(o_sp[:], ps_o[:], AF.Copy)
                omix[(k, "sp")] = o_sp
                dmix[(k, "pw")] = par_mix(wpw[(k, 0)][:], wpw[(k, 1)][:],
                                          NPAR, PAR, f"dpw_{k}", [C, 2],
                                          "wbu")
                omix[(k, "pw")] = par_mix(wpw[(k, 0)][:], wpw[(k, 1)][:],
                                          PAR, NPAR, f"crpw_{k}", [C, 2],
                                          "wbu")

            lgt = {}
            ldt = {}

            def weight_prep(jb, k):
                # grouped lhsT, unscaled (scale by s_in post-AR, in place)
                d1 = sb.tile([C, 9, C], DT, name=f"d1_{jb}{k}", tag="dsel",
                             bufs=2)
                nc.gpsimd.affine_select(
                    d1[:], dmix[(k, "sp")][:, jb, :].unsqueeze(2)
                    .broadcast_to([C, 9, C]),
                    pattern=[[0, 9], [1, C]], compare_op=ALU.is_equal,
                    fill=0.0, base=0, channel_multiplier=-1)
                d2 = sb.tile([C, 9, C], DT, name=f"d2_{jb}{k}", tag="dsel",
                             bufs=2)
                nc.gpsimd.affine_select(
                    d2[:], omix[(k, "sp")][:, jb, :].unsqueeze(2)
                    .broadcast_to([C, 9, C]),
                    pattern=[[0, 9], [1, C]], compare_op=ALU.is_equal,
                    fill=0.0, base=0, channel_multiplier=-1)
                lg = sb.tile([C, 9, 64, 2], F32R, name=f"lg_{jb}{k}",
                             tag=f"lhsg_{k}", bufs=2)
                nc.vector.tensor_tensor(
                    lg[:],
                    d1[:].rearrange("p t (a b) -> p t a b", b=2),
                    d2[:].rearrange("p t (a b) -> p t a b", b=2)[:, :, :, ::-1],
                    op=ALU.add)
                lgt[(jb, k)] = lg

                # dense lhsT = (CW . PW) pre-transposed; PW off-diag is the
                # unswapped cross mix
                p1 = sb.tile([C, C], DT, name=f"p1_{jb}{k}", tag="psel",
                             bufs=2)
                nc.gpsimd.affine_select(
                    p1[:], dmix[(k, "pw")][:, jb:jb + 1].broadcast_to([C, C]),
                    pattern=[[1, C]], compare_op=ALU.is_equal, fill=0.0,
                    base=0, channel_multiplier=-1)
                p2 = sb.tile([C, C], DT, name=f"p2_{jb}{k}", tag="psel",
                             bufs=2)
                nc.gpsimd.affine_select(
                    p2[:], omix[(k, "pw")][:, jb:jb + 1].broadcast_to([C, C]),
                    pattern=[[1, C]], compare_op=ALU.is_equal, fill=0.0,
                    base=0, channel_multiplier=-1)
                pw_t = sb.tile([C, 64, 2], F32R, name=f"pwm_{jb}{k}",
                               tag="pwm", bufs=2)
                nc.vector.tensor_tensor(
                    pw_t[:],
                    p1[:].rearrange("p (a b) -> p a b", b=2),
                    p2[:].rearrange("p (a b) -> p a b", b=2)[:, :, ::-1],
                    op=ALU.add)
                ld = sb.tile([C, 9, C], F32R, name=f"ld_{jb}{k}",
                             tag=f"lhsd_{k}", bufs=2)
                for t in range(9):
                    ps_w = psp.tile([C, C], DT, name=f"psw_{jb}{k}{t}",
                                    tag="ps", bufs=7)
                    nc.tensor.matmul(
                        ps_w[:], pw_t[:].rearrange("p a b -> p (a b)"),
                        cwt_t[k][:, t, :], start=True, stop=True)
                    nc.scalar.activation(ld[:, t, :], ps_w[:], AF.Copy)
                ldt[(jb, k)] = ld

            # ---------- post-AR finalize: s_in, scale lg in place ----------
            s_in_t = {}

            def finalize(jb, gs):
                for ki, k in enumerate("hl"):
                    g = geo[k]
                    n_img = (4 * g["orows"]) * g["w"]
                    col = 2 * ki
                    m = sb.tile([C, 1], DT, name=f"m_{jb}{k}", tag="wb1",
                                bufs=8)
                    nc.scalar.activation(m[:], gs[:, col:col + 1], AF.Copy,
                                         scale=1.0 / n_img)
                    ex2 = sb.tile([C, 1], DT, name=f"ex2_{jb}{k}", tag="wb1",
                                  bufs=8)
                    nc.scalar.activation(ex2[:], gs[:, col + 1:col + 2],
                                         AF.Copy, scale=1.0 / n_img)
                    var = sb.tile([C, 1], DT, name=f"var_{jb}{k}", tag="wb1",
                                  bufs=8)
                    nc.vector.tensor_tensor(var[:], m[:], m[:], op=ALU.mult)
                    nc.vector.tensor_tensor(var[:], ex2[:], var[:],
                                            op=ALU.subtract)
                    sd = sb.tile([C, 1], DT, name=f"sd_{jb}{k}", tag="wb1",
                                 bufs=8)
                    nc.scalar.activation(sd[:], var[:], AF.Sqrt, bias=epst[:])
                    s_in = sb.tile([C, 1], DT, name=f"sin_{jb}{k}", tag="wb1",
                                   bufs=8)
                    nc.vector.reciprocal(s_in[:], sd[:])
                    s_in_t[(jb, k)] = s_in
                    lg = lgt[(jb, k)]
                    nc.vector.tensor_scalar_mul(lg[:], lg[:], s_in[:])

            # ---------- conv for one (job, branch) ----------
            def conv_branch(jb, k):
                g = geo[k]
                w, wp, nblk = g["w"], g["wp"], g["nblk"]
                lg, ld = lgt[(jb, k)], ldt[(jb, k)]
                nstrip = nblk * (16 // (512 // w))
                sumparts = sb.tile([C, nstrip], DT, name=f"osum_{jb}{k}")
                sqparts = sb.tile([C, nstrip], DT, name=f"osq_{jb}{k}")
                stage = None
                for blk in range(nblk):
                    xb = sb.tile([C, 20, wp], F32R, name=f"xb_{jb}{k}{blk}",
                                 tag="xb", bufs=2)
                    nc.sync.dma_start(
                        out=xb[:],
                        in_=xin[k].ap()[jb, :, 16 * blk:16 * blk + 20, :])
                    yb = sb.tile([C, 18, wp], F32R, name=f"yb_{jb}{k}{blk}",
                                 tag="yb", bufs=2)
                    gst = []  # grouped strips: (row0, nrows)
                    r = 0
                    while r < 18:
                        n = min(512 // w, 18 - r)
                        gst.append((r, n))
                        r += n
                    for (r0, nr) in gst:
                        ps_g = psp.tile([C, nr, w], DT,
                                        name=f"psg_{jb}{k}{blk}{r0}",
                                        tag="ps", bufs=7)
                        for t in range(9):
                            dy, dx = divmod(t, 3)
                            dy -= 1
                            nc.tensor.matmul(
                                ps_g[:], lg[:].rearrange(
                                    "p t a b -> p t (a b)")[:, t, :],
                                xb[:, r0 + 1 + dy:r0 + 1 + nr + dy,
                                   dx:dx + w],
                                start=(t == 0), stop=(t == 8))
                        nc.scalar.activation(yb[:, r0:r0 + nr, 1:1 + w],
                                             ps_g[:], AF.Copy)
                        nc.vector.tensor_copy(
                            yb[:, r0:r0 + nr, 0:wp:wp - 1],
                            yb[:, r0:r0 + nr, 2:w:w - 3])
                    if blk == 0:
                        e1 = sb.tile([C, 1, wp], DT, name=f"e1_{jb}{k}",
                                     tag="edge", bufs=2)
                        nc.vector.tensor_scalar_mul(e1[:], yb[:, 0:1, :],
                                                    cvec[:, 3:4])
                        e2 = sb.tile([C, 1, wp], DT, name=f"e2_{jb}{k}",
                                     tag="edge", bufs=2)
                        nc.vector.tensor_scalar_mul(e2[:], yb[:, 2:3, :],
                                                    cvec[:, 2:3])
                        nc.vector.tensor_tensor(yb[:, 0:1, :], e1[:], e2[:],
                                                op=ALU.add)
                    if blk == nblk - 1:
                        e3 = sb.tile([C, 1, wp], DT, name=f"e3_{jb}{k}",
                                     tag="edge", bufs=2)
                        nc.vector.tensor_scalar_mul(e3[:], yb[:, 17:18, :],
                                                    cvec[:, 5:6])
                        e4 = sb.tile([C, 1, wp], DT, name=f"e4_{jb}{k}",
                                     tag="edge", bufs=2)
                        nc.vector.tensor_scalar_mul(e4[:], yb[:, 15:16, :],
                                                    cvec[:, 4:5])
                        nc.vector.tensor_tensor(yb[:, 17:18, :], e3[:], e4[:],
                                                op=ALU.add)
                    rps = 512 // w  # out rows per dense strip
                    nds = 16 // rps  # dense strips per block
                    spst = 8 // rps  # strips per 8-row stage
                    for d in range(nds):
                        sidx = nds * blk + d
                        if d % spst == 0:
                            stage = sb.tile([C, 8, w], BF16,
                                            name=f"stg_{jb}{k}{blk}{d}",
                                            tag="stage", bufs=3)
                        ps_d = psp.tile([C, rps, w], DT,
                                        name=f"psd_{jb}{k}{blk}{d}",
                                        tag="ps", bufs=7)
                        for t in range(9):
                            dy, dx = divmod(t, 3)
                            dy -= 1
                            nc.tensor.matmul(
                                ps_d[:], ld[:, t, :],
                                yb[:, rps * d + 1 + dy:rps * (d + 1) + 1 + dy,
                                   dx:dx + w],
                                start=(t == 0), stop=(t == 8))
                        nc.scalar.activation(
                            stage[:, rps * (d % spst):rps * (d % spst + 1), :],
                            ps_d[:], AF.Copy,
                            accum_out=sumparts[:, sidx:sidx + 1])
                        nc.scalar.activation(
                            sqsc[:, 0:rps * w], ps_d[:], AF.Square,
                            accum_out=sqparts[:, sidx:sidx + 1])
                        if d % spst == spst - 1:
                            r0 = 16 * blk + rps * (d - spst + 1)
                            nc.sync.dma_start(
                                out=op_scr[(jb, k)][:, r0:r0 + 8, :],
                                in_=stage[:])
                # out-stats AR for this (job, branch)
                ar2 = sb.tile([C, 2], DT, name=f"ar2in_{jb}{k}")
                nc.vector.tensor_reduce(ar2[:, 0:1], sumparts[:], axis=AX.X,
                                        op=ALU.add)
                nc.vector.tensor_reduce(ar2[:, 1:2], sqparts[:], axis=AX.X,
                                        op=ALU.add)
                bi = dr.tile([C, 2], DT, name=f"ar2bi_{jb}{k}")
                bo = dr.tile([C, 2], DT, name=f"ar2bo_{jb}{k}")
                nc.gpsimd.dma_start(out=bi[:], in_=ar2[:])
                nc.gpsimd.collective_compute(
                    "AllReduce", ALU.add, replica_groups=GROUPS,
                    ins=[bi[:].opt()], outs=[bo[:].opt()])
                go = sb.tile([C, 2], DT, name=f"gout_{jb}{k}")
                nc.gpsimd.dma_start(out=go[:], in_=bo[:])
                return go

            # ---------- epilogue ----------
            def epilogue(jb, k, go):
                g = geo[k]
                w = g["w"]
                n_img = (4 * g["orows"]) * w
                m = sb.tile([C, 1], DT, name=f"em_{jb}{k}", tag="wb1", bufs=8)
                nc.scalar.activation(m[:], go[:, 0:1], AF.Copy,
                                     scale=1.0 / n_img)
                ex2 = sb.tile([C, 1], DT, name=f"eex2_{jb}{k}", tag="wb1",
                              bufs=8)
                nc.scalar.activation(ex2[:], go[:, 1:2], AF.Copy,
                                     scale=1.0 / n_img)
                var = sb.tile([C, 1], DT, name=f"evar_{jb}{k}", tag="wb1",
                              bufs=8)
                nc.vector.tensor_tensor(var[:], m[:], m[:], op=ALU.mult)
                nc.vector.tensor_tensor(var[:], ex2[:], var[:],
                                        op=ALU.subtract)
                sd = sb.tile([C, 1], DT, name=f"esd_{jb}{k}", tag="wb1",
                             bufs=8)
                nc.scalar.activation(sd[:], var[:], AF.Sqrt, bias=epst[:])
                sc = sb.tile([C, 1], DT, name=f"esc_{jb}{k}", tag="wb1",
                             bufs=8)
                nc.vector.reciprocal(sc[:], sd[:])
                bi = sb.tile([C, 1], DT, name=f"ebi_{jb}{k}", tag="wb1",
                             bufs=8)
                nc.vector.tensor_tensor(bi[:], m[:], sc[:], op=ALU.mult)
                nc.vector.tensor_scalar_mul(bi[:], bi[:], -1.0)
                for r0 in range(0, g["orows"], 8):
                    ei = sb.tile([C, 8, w], BF16, name=f"ei_{jb}{k}{r0}",
                                 tag="epin", bufs=2)
                    nc.sync.dma_start(out=ei[:],
                                      in_=op_scr[(jb, k)][:, r0:r0 + 8, :])
                    eo = sb.tile([C, 8, w], DT, name=f"eo_{jb}{k}{r0}",
                                 tag="epout", bufs=2)
                    nc.scalar.activation(eo[:], ei[:], AF.Prelu, bias=bi[:],
                                         scale=sc[:], alpha=alphat[:])
                    nc.sync.dma_start(out=out_d[k].ap()[jb, :, r0:r0 + 8, :],
                                      in_=eo[:])

            # ================= schedule =================
            gsA = in_stats(0)
            predictor("h")
            predictor("l")
            for k in "hl":
                mixes(k)
            for jb in range(2):
                for k in "hl":
                    weight_prep(jb, k)
            finalize(0, gsA)
            gsB = in_stats(1)
            finalize(1, gsB)
            gout = {}
            gout[(0, "h")] = conv_branch(0, "h")
            gout[(1, "h")] = conv_branch(1, "h")
            gout[(0, "l")] = conv_branch(0, "l")
            gout[(1, "l")] = conv_branch(1, "l")
            for jb, k in [(0, "h"), (1, "h"), (0, "l"), (1, "l")]:
                epilogue(jb, k, gout[(jb, k)])

    nc.finalize()
    return nc


@functools.lru_cache(maxsize=1)
def _graph():
    return _build()


def _prep_host(inputs):
    c_hf = np.asarray(inputs["c_hf"], np.float32)
    c_lf = np.asarray(inputs["c_lf"], np.float32)
    s_hf = np.asarray(inputs["s_hf"], np.float32)
    s_lf = np.asarray(inputs["s_lf"], np.float32)

    def wprep(sp_w, pw_w, sp_b, pw_b, cw):
        w = sp_w.reshape(C, 2, SC, 9)
        spw_a = np.ascontiguousarray(
            np.transpose(w.reshape(C, 2, 4, C, 9), (4, 1, 2, 3, 0)))
        pw = pw_w.reshape(C, 2, SC)
        pww_a = np.ascontiguousarray(
            np.transpose(pw.reshape(C, 2, 4, C), (1, 2, 3, 0)))
        spb_a = np.ascontiguousarray(sp_b.reshape(C, 2))
        pwb_a = np.ascontiguousarray(pw_b.reshape(C, 2))
        cwt_a = np.ascontiguousarray(
            cw.reshape(C, C, 9).transpose(1, 2, 0))
        return spw_a, pww_a, spb_a, pwb_a, cwt_a

    wh = wprep(np.asarray(inputs["kp_h_sp_w"], np.float32),
               np.asarray(inputs["kp_h_pw_w"], np.float32),
               np.asarray(inputs["kp_h_sp_b"], np.float32),
               np.asarray(inputs["kp_h_pw_b"], np.float32),
               np.asarray(inputs["conv_h_w"], np.float32))
    wl = wprep(np.asarray(inputs["kp_l_sp_w"], np.float32),
               np.asarray(inputs["kp_l_pw_w"], np.float32),
               np.asarray(inputs["kp_l_sp_b"], np.float32),
               np.asarray(inputs["kp_l_pw_b"], np.float32),
               np.asarray(inputs["conv_l_w"], np.float32))

    pswap = np.zeros((C, C), np.float32)
    for c in range(C):
        pswap[c ^ 1, c] = 1.0

    xpad_h = [np.pad(c_hf[b], ((0, 0), (2, 2), (1, 1)), "reflect")
              for b in range(B)]
    xpad_l = [np.pad(c_lf[b], ((0, 0), (2, 2), (1, 1)), "reflect")
              for b in range(B)]
    def s_im2col(s01):
        # s01 [2, SC, 3, 3] -> [SC, 9, 2, 9]: [ci, tap, b, pix]
        sp = np.stack([np.pad(s, ((0, 0), (1, 1), (1, 1)), "reflect")
                       for s in s01])  # [2, SC, 5, 5]
        out = np.empty((SC, 9, 2, 9), np.float32)
        for t in range(9):
            ty, tx = divmod(t, 3)
            out[:, t] = np.transpose(
                sp[:, :, ty:ty + 3, tx:tx + 3].reshape(2, SC, 9), (1, 0, 2))
        return np.ascontiguousarray(out)

    in_maps = []
    for i in range(N_CORES):
        g, q = i // 4, i % 4
        bs = (2 * g, 2 * g + 1)
        cvec = np.zeros((C, 16), np.float32)
        cvec[1::2, 0] = 1.0          # parity
        cvec[:, 1] = 1.0 - cvec[:, 0]
        cvec[:, 2] = 1.0 if q == 0 else 0.0
        cvec[:, 3] = 1.0 - cvec[:, 2]
        cvec[:, 4] = 1.0 if q == 3 else 0.0
        cvec[:, 5] = 1.0 - cvec[:, 4]
        cvec[:, 6:8] = wh[2]
        cvec[:, 8:10] = wh[3]
        cvec[:, 10:12] = wl[2]
        cvec[:, 12:14] = wl[3]
        m = {
            "x_hf": np.ascontiguousarray(
                np.stack([xpad_h[b][:, 64 * q:64 * q + 68, :] for b in bs])),
            "x_lf": np.ascontiguousarray(
                np.stack([xpad_l[b][:, 32 * q:32 * q + 36, :] for b in bs])),
            "s_hf": s_im2col(s_hf[list(bs)]),
            "s_lf": s_im2col(s_lf[list(bs)]),
            "spw_h": wh[0], "pww_h": wh[1], "cwT_h": wh[4],
            "spw_l": wl[0], "pww_l": wl[1], "cwT_l": wl[4],
            "pswap": pswap, "cvec": cvec,
        }
        in_maps.append(m)
    return in_maps


def kernel(_trace=False, **inputs):
    global LAST_EXEC_NS
    from concourse.bass_utils import run_bass_kernel_spmd

    nc = _graph()
    in_maps = _prep_host(inputs)
    res = run_bass_kernel_spmd(nc, in_maps, core_ids=list(range(N_CORES)),
                               trace=_trace)
    LAST_EXEC_NS = res.exec_time_ns

    oh = np.zeros((B, C, H, H), np.float32)
    ol = np.zeros((B, C, H // 2, H // 2), np.float32)
    for i in range(N_CORES):
        g, q = i // 4, i % 4
        r = res.results[i]
        for jb, b in enumerate((2 * g, 2 * g + 1)):
            oh[b, :, 64 * q:64 * (q + 1), :] = r["out_hf"][jb]
            ol[b, :, 32 * q:32 * (q + 1), :] = r["out_lf"][jb]
    return oh, ol
